# revision 21
# baseline (speedup 1.0000x reference)
"""Trainium2 Bass kernel for nn_AdvancedIFTransformerClassifier.

Self-contained: takes FULL inputs (as from setup_inputs()), shards batch
B=512 across 8 NeuronCores (64 samples each, pure data parallel), runs one
compiled Bass/Tile program per core, gathers [512, 100] output.

Design notes:
- All GEMMs fp32r (FP22 multiply, fp32 accumulate) at full PE rate; FFN
  weights + activations bf16 (same PE rate, half DMA/SBUF).
- Activations channel-major [C, (b,t)] in SBUF; FFTs are DFT-matrix matmuls.
- Everything tiled in 500-token tiles (= 4 attention chunks of 5 samples,
  = 20-sample conv blocks); LN stats/apply software-pipelined one tile
  behind the producing GEMM phase so the PE queue never stalls.
- LN rstd on DVE only (fast-inverse-sqrt + 2 Newton steps): no ACT table
  loads on the LN path (tables switch only for softmax exp / FFN gelu).
- Attention: qkv(c+1) | mid-logits(c) | mid-out(c-1) pipeline; attention
  output is produced channel-major via (vs.T @ D) matmuls - no transposes.
- Convs (stage0 + SSM) read shifted windows via strided matmul rhs APs;
  SSM accumulates all 3 taps in one PSUM group.
"""
import sys
import types
import numpy as np
from contextlib import ExitStack


def _install_ntff_hook():
    try:
        import antenv.axon_hooks  # noqa: F401
        return
    except ImportError:
        pass
    try:
        from trn_agent_boot.trn_boot import _ntff_profile_via_ctypes
        hook = _ntff_profile_via_ctypes('/opt/axon/libaxon_pjrt.so')
    except Exception:
        hook = None
    mod = types.ModuleType('antenv.axon_hooks')
    mod._hook = hook
    mod.get_axon_ntff_profile_hook = lambda: mod._hook
    mod.set_axon_ntff_profile_hook = lambda h: setattr(mod, '_hook', h)
    sys.modules['antenv.axon_hooks'] = mod


_install_ntff_hook()

import concourse.bass as bass  # noqa: E402
import concourse.tile as tile  # noqa: E402
from concourse import bacc, mybir  # noqa: E402
from concourse.bass_utils import run_bass_kernel_spmd  # noqa: E402

FP32 = mybir.dt.float32
FP32R = mybir.dt.float32r
BF16 = mybir.dt.bfloat16
U16 = mybir.dt.uint16
U32 = mybir.dt.uint32
AF = mybir.ActivationFunctionType
ALU = mybir.AluOpType
AX = mybir.AxisListType

# ---- problem dims (hardcoded) ----
B, T, DIN = 512, 96, 64
H, PATCH, L, HEADS, NCLS = 256, 2, 4, 8, 100
E = 3 * H                    # 768
T2 = T // PATCH              # 48
F = T2 // 2 + 1              # 25 (transformer seq len)
FQ = F // 2 + 1              # 13
DH = E // HEADS              # 96
EPS = 1e-5
NCORES = 8
BL = B // NCORES             # 64 samples/core
TOKS = BL * F                # 1600 transformer tokens/core
NCH = E // 128               # 6 channel chunks
WZ = 1800                    # zb per-chunk width (>= 1+BL*28 = 1793)
WH = 1600                    # h per-chunk width

# 500-token tiles (chunk-aligned; tile t covers chunks 4t..4t+3, tile 3 = chunk 12)
T500 = [(0, 500), (500, 500), (1000, 500), (1500, 100)]
# b-aligned blocks (conv / ssm / reduce): (b_off, nb) -- same token ranges as T500
BT4 = [(0, 20), (20, 20), (40, 20), (60, 4)]
# attention chunks: (tok_off, nb)
CHUNKS = [(i * 125, 5) for i in range(12)] + [(1500, 4)]
TILE_CHUNKS = [CHUNKS[0:4], CHUNKS[4:8], CHUNKS[8:12], CHUNKS[12:13]]

RSQRT_MAGIC = 0x5f3759df


def _np_consts():
    t2 = np.arange(T2)[:, None]
    f = np.arange(F)[None, :]
    C48 = np.cos(2 * np.pi * t2 * f / T2)                      # [48, 25]
    c48bd = np.zeros((96, 50))
    c48bd[0:48, 0:25] = C48
    c48bd[48:96, 25:50] = C48

    tt = np.arange(F)[:, None]
    fq = np.arange(FQ)[None, :]
    C25r = np.cos(2 * np.pi * tt * fq / F)                     # [25, 13]
    C25i = -np.sin(2 * np.pi * tt * fq / F)
    a = np.full(FQ, 2.0 / F)
    a[0] = 1.0 / F
    Dr = a[:, None] * np.cos(2 * np.pi * np.arange(FQ)[:, None] * np.arange(F)[None, :] / F)
    Di = -a[:, None] * np.sin(2 * np.pi * np.arange(FQ)[:, None] * np.arange(F)[None, :] / F)

    sc = DH ** -0.25

    def blockdiag(m, nb):
        r, c = m.shape
        out = np.zeros((r * nb, c * nb))
        for i in range(nb):
            out[i * r:(i + 1) * r, i * c:(i + 1) * c] = m
        return out

    cst = {}
    for nb, sfx in ((5, "5"), (4, "4")):
        cst["cqk_r" + sfx] = blockdiag(C25r * sc, nb)          # [125|100, 65|52]
        cst["cqk_i" + sfx] = blockdiag(C25i * sc, nb)
        cst["cv_r" + sfx] = blockdiag(C25r, nb)
        cst["cv_i" + sfx] = blockdiag(C25i, nb)
        ntp = nb * 25 + (nb % 2)                               # even-padded token count
        dr = np.zeros((FQ * nb, ntp)); dr[:, 0:nb * 25] = blockdiag(Dr, nb)
        di = np.zeros((FQ * nb, ntp)); di[:, 0:nb * 25] = blockdiag(Di, nb)
        cst["d_r" + sfx] = dr                                  # [65|52, 126|100]
        cst["d_i" + sfx] = di
    cst["c48bd"] = c48bd
    cst["eye"] = np.eye(128)
    cst["inv_e"] = np.full((128, 1), 1.0 / E)
    cst["ones_row"] = np.ones((1, 512))
    return {k: v.astype(np.float32) for k, v in cst.items()}


def _r22(x):
    """Round to nearest FP22 so the PE's fp32r truncation becomes exact."""
    u = (np.ascontiguousarray(x, np.float32).view(np.uint32) + (1 << 9)) & np.uint32(0xFFFFFC00)
    return u.view(np.float32)


def _bf16(x):
    """fp32 -> bf16 bits (round to nearest even) as uint16."""
    u = np.ascontiguousarray(x, np.float32).view(np.uint32)
    u = (u + 0x7FFF + ((u >> 16) & 1)) >> 16
    return u.astype(np.uint16)


def _pp(v, nch):
    """[C] -> [128, nch] per-partition layout (col j = chunk j)."""
    return np.ascontiguousarray(v.reshape(nch, 128).T).astype(np.float32)


def _prep(inputs):
    """Host-side prep: returns (shared_map, per-core x list)."""
    g = {k: np.asarray(v, dtype=np.float32) for k, v in inputs.items()}
    cst = _np_consts()
    sh = dict(cst)

    sh["w_in"] = g["W_in"]                                      # [64, 256]
    # fold b_in into pe (broadcast over t)
    sh["pe_eff"] = np.ascontiguousarray(
        (g["pe"][0, :T, :] + g["b_in"][None, :]).T)             # [256, 96]
    sh["w_shape"] = g["W_shape"]                                # [256, 256]
    sh["b_shape_pp"] = _pp(g["b_shape"], 2)
    sh["w_patch"] = g["W_patch"]                                # [512, 256]
    sh["b_patch_row"] = g["b_patch"][None, :]                   # [1, 256]
    # conv taps [7, 256, 256] in (cin, cout) layout; order: w1k0, w2k0,w2k1, w4k0..3
    taps = []
    for wname in ("conv_w1", "conv_w2", "conv_w4"):
        w = g[wname]                                            # [O, I, k]
        for kk in range(w.shape[2]):
            taps.append(np.ascontiguousarray(w[:, :, kk].T))
    sh["convw"] = np.stack(taps)                                # [7, 256, 256]
    sh["conv_b_pp"] = _pp(np.concatenate([g["conv_b1"], g["conv_b2"], g["conv_b4"]]), 6)

    sh["wqkv"] = g["Wqkv"]                                      # [4, 768, 2304]
    sh["wo"] = g["Wo"]                                          # [4, 768, 768]
    sh["bo_pp"] = np.concatenate([_pp(g["bo"][i], 6) for i in range(L)], 1)    # [128, 24]
    sh["bf1_pp"] = np.concatenate([_pp(g["bf1"][i], 24) for i in range(L)], 1)  # [128, 96]
    sh["bf2_pp"] = np.concatenate([_pp(g["bf2"][i], 6) for i in range(L)], 1)
    for nm in ("ln1_g", "ln1_b", "ln2_g", "ln2_b"):
        sh[nm + "_pp"] = np.concatenate([_pp(g[nm][i], 6) for i in range(L)], 1)
    # ssm taps [3, 768, 768] (cin, cout)
    sh["ssmw"] = np.stack([np.ascontiguousarray(g["ssm_w"][:, :, kk].T) for kk in range(3)])
    sh["ssmb_pp"] = _pp(g["ssm_b"], 6)
    sh["ssmg_pp"] = _pp(g["ssm_g"], 6)
    sh["ssmbn25_pp"] = _pp(F * g["ssm_bn"], 6)
    # fold final-LN gamma/beta and the 1/F mean into the head weights (exact):
    # out = (gamma*(h-m)*rstd + beta) . Wout/F  ==  wog.T @ G - uvec x S_m + bout2
    wog = g["ssm_g"][:, None] * (g["W_out"] / F)                # [768, 100]
    sh["wout"] = wog
    sh["neguv"] = -np.sum(wog, axis=0)[None, :]                 # [1, 100]
    sh["bout_pp"] = (g["b_out"] + g["W_out"].T @ g["ssm_bn"])[:, None]

    x = g["x"]                                                  # [512, 96, 64]
    xs = []
    for c in range(NCORES):
        xc = x[c * BL:(c + 1) * BL]                             # [64, 96, 64]
        xs.append(np.ascontiguousarray(xc.transpose(2, 0, 1).reshape(DIN, BL * T)))
    sh["wf1"] = g["Wf1"]                                        # [4, 768, 3072]
    sh["wf2"] = g["Wf2"]                                        # [4, 3072, 768]
    sh = {k: _r22(np.ascontiguousarray(v, dtype=np.float32)) for k, v in sh.items()}
    xs = [_r22(v) for v in xs]
    return sh, xs


# ---------------------------------------------------------------------------
def _build():
    nc = bacc.Bacc("TRN2", target_bir_lowering=False, debug=False, num_devices=NCORES)

    def din(name, shape, dt=FP32):
        return nc.dram_tensor(name, list(shape), dt, kind="ExternalInput")

    d = {}
    d["xcm"] = din("xcm", [DIN, BL * T])
    d["w_in"] = din("w_in", [DIN, H])
    d["pe_eff"] = din("pe_eff", [H, T])
    d["w_shape"] = din("w_shape", [H, H])
    d["b_shape_pp"] = din("b_shape_pp", [128, 2])
    d["w_patch"] = din("w_patch", [2 * H, H])
    d["b_patch_row"] = din("b_patch_row", [1, H])
    d["convw"] = din("convw", [7, H, H])
    d["conv_b_pp"] = din("conv_b_pp", [128, 6])
    d["wqkv"] = din("wqkv", [L, E, 3 * E])
    d["wo"] = din("wo", [L, E, E])
    d["bo_pp"] = din("bo_pp", [128, 6 * L])
    d["wf1"] = din("wf1", [L, E, 4 * E])
    d["bf1_pp"] = din("bf1_pp", [128, 24 * L])
    d["wf2"] = din("wf2", [L, 4 * E, E])
    d["bf2_pp"] = din("bf2_pp", [128, 6 * L])
    for nm in ("ln1_g", "ln1_b", "ln2_g", "ln2_b"):
        d[nm + "_pp"] = din(nm + "_pp", [128, 6 * L])
    d["ssmw"] = din("ssmw", [3, E, E])
    for nm in ("ssmb_pp", "ssmg_pp", "ssmbn25_pp"):
        d[nm] = din(nm, [128, 6])
    d["wout"] = din("wout", [E, NCLS])
    d["neguv"] = din("neguv", [1, NCLS])
    d["bout_pp"] = din("bout_pp", [NCLS, 1])
    for nm, shp in (("c48bd", [96, 50]), ("eye", [128, 128]),
                    ("inv_e", [128, 1]), ("ones_row", [1, 512])):
        d[nm] = din(nm, shp)
    for sfx, r, rp, c in (("5", 125, 126, 65), ("4", 100, 100, 52)):
        for nm in ("cqk_r", "cqk_i", "cv_r", "cv_i"):
            d[nm + sfx] = din(nm + sfx, [r, c])
        for nm in ("d_r", "d_i"):
            d[nm + sfx] = din(nm + sfx, [c, rp])
    out_d = nc.dram_tensor("out", [BL, NCLS], FP32, kind="ExternalOutput")

    with tile.TileContext(nc) as tc, ExitStack() as ctx:
        _program(nc, tc, ctx, d, out_d)
    nc.compile()
    return nc


def _program(nc, tc, ctx, d, out_d):
    V, S = nc.vector, nc.scalar

    ps = ctx.enter_context(tc.tile_pool(name="ps", bufs=1, space="PSUM"))
    cst = ctx.enter_context(tc.tile_pool(name="cst", bufs=1))
    per = ctx.enter_context(tc.tile_pool(name="per", bufs=1))
    tmp = ctx.enter_context(tc.tile_pool(name="tmp", bufs=1))

    def pst(shape, tag, bufs):
        return ps.tile(shape, FP32, tag=tag, bufs=bufs, name=f"ps_{tag}")

    # PSUM tags (8 banks): rot=2 (big GEMMs), aux=2 (mid DFT + patch),
    # t=1 (transposes), sb=3 (LN stats tile + apply broadcast pair)

    # ---- persistent activation buffers ----
    h = per.tile([128, NCH * WH], FP32R, tag="h", name="h")        # h chunks, stride WH
    zb = per.tile([128, NCH * WZ], FP32R, tag="zb", name="zb")     # z / o / h_pad
    hmean = per.tile([128, NCH * BL], FP32R, tag="hmean", name="hmean")

    # ---- constants in SBUF (stage0-critical first; attention consts later) ----
    def cload(name, shape, dt=FP32R, src=None):
        t = cst.tile(list(shape), dt, tag=name, name=name)
        ap = (src if src is not None else d[name][:, :])
        nc.sync.dma_start(t[0:shape[0], 0:shape[1]],
                          ap.bitcast(dt) if dt == FP32R else ap)
        return t

    eye = cload("eye", [128, 128], FP32)
    inv_e = cload("inv_e", [128, 1])
    ones_row = cst.tile([33, 512], FP32R, tag="ones_row", name="ones_row")
    nc.sync.dma_start(ones_row[0:1, 0:512], d["ones_row"][:, :].bitcast(FP32R))
    nc.sync.dma_start(ones_row[32:33, 0:512], d["ones_row"][:, :].bitcast(FP32R))
    c48bd = cload("c48bd", [96, 50])
    b_patch_row = cload("b_patch_row", [1, H])
    pp = {}
    for nm in ("b_shape_pp", "conv_b_pp", "bo_pp", "bf1_pp", "bf2_pp",
               "ln1_g_pp", "ln1_b_pp", "ln2_g_pp", "ln2_b_pp",
               "ssmb_pp", "ssmg_pp", "ssmbn25_pp"):
        shp = list(d[nm].shape)
        pp[nm] = cst.tile(shp, FP32, tag=nm, name=nm)
        nc.sync.dma_start(pp[nm][:, :], d[nm][:, :])

    # ---- small reusable tmp tags ----
    def ttile(shape, tag, bufs, dt=FP32):
        return tmp.tile(list(shape), dt, tag=tag, bufs=bufs, name=f"t_{tag}")

    # =======================================================================
    # LayerNorm helpers (per 500-token tile, PE stats + DVE-only rsqrt)
    # =======================================================================
    def stats_tile(toff, n):
        """mean(h) -> sb psum row 0, mean(h^2) -> aux psum row 0."""
        sbt = pst([128, 512], "sb", 3)
        sbt2 = pst([128, 512], "aux", 2)
        for ci in range(NCH):
            sl = h[:, ci * WH + toff: ci * WH + toff + n]
            nc.tensor.matmul(sbt[0:1, 0:n], inv_e[:, :], sl,
                             start=(ci == 0), stop=(ci == NCH - 1))
            sq = ttile([128, 512], "u6", 2, FP32R)
            S.activation(sq[:, 0:n], sl.bitcast(FP32), AF.Square)
            nc.tensor.matmul(sbt2[0:1, 0:n], inv_e[:, :], sq[:, 0:n],
                             start=(ci == 0), stop=(ci == NCH - 1))
        return sbt, sbt2

    def chain_tile(sbt, sbt2, n):
        """rows cols [0:512] = rstd, cols [512:1024] = mean*rstd (partition 0).

        DVE only (no ACT tables): fast-inverse-sqrt seed + 2 Newton steps.
        All operands live on partition 0 (TensorTensor requires matching
        input start partitions, and at most one PSUM input).
        """
        rows = ttile([1, 1024], "rows", 3, FP32R)   # a | m*a (final, fp32r)
        rsc = ttile([1, 2048], "rsc", 1)            # y | t1 | var | m (fp32)
        y, t1 = rsc[0:1, 0:n], rsc[0:1, 512:512 + n]
        var, m_s = rsc[0:1, 1024:1024 + n], rsc[0:1, 1536:1536 + n]
        V.tensor_copy(m_s, sbt[0:1, 0:n])           # mean
        V.tensor_tensor(t1, m_s, m_s, ALU.mult)
        V.scalar_tensor_tensor(var, sbt2[0:1, 0:n], EPS, t1, ALU.add, ALU.subtract)
        y_u, var_u = y.bitcast(U32), var.bitcast(U32)
        V.tensor_scalar(y_u, var_u, 1, None, ALU.logical_shift_right)
        # magic - x, computed in the ALU's float domain (seed precision is
        # irrelevant; both values < 2^31 so no wraparound is needed)
        V.tensor_scalar(y_u, y_u, -1.0, float(RSQRT_MAGIC), ALU.mult, ALU.add)
        for _ in range(1):  # Newton: y *= 1.5 - 0.5*var*y*y  (1 iter: ~2e-3 rel)
            V.tensor_tensor(t1, y, y, ALU.mult)
            V.tensor_tensor(t1, t1, var, ALU.mult)
            V.tensor_scalar(t1, t1, -0.5, 1.5, ALU.mult, ALU.add)
            V.tensor_tensor(y, y, t1, ALU.mult)
        V.tensor_copy(rows[0:1, 0:n], y)
        V.tensor_tensor(rows[0:1, 512:512 + n], m_s, y, ALU.mult)
        return rows

    def ln_begin(toff, n):
        return chain_tile(*stats_tile(toff, n), n)

    def zslice(ci, toff, n):        # LN1 z (fp32r, zb strided WZ)
        return zb[:, ci * WZ + toff: ci * WZ + toff + n]

    def bcast_rows(rows, n):
        ap_ps = pst([128, 512], "sb", 3)
        nc.tensor.matmul(ap_ps[:, 0:n], ones_row[0:1, 0:128],
                         rows[0:1, 0:n], start=True, stop=True)
        mb_ps = pst([128, 512], "sb", 3)
        nc.tensor.matmul(mb_ps[:, 0:n], ones_row[0:1, 0:128],
                         rows[0:1, 512:512 + n], start=True, stop=True)
        return ap_ps, mb_ps

    def apply_tile(dst_slice, rows, gname, bname, pcol, toff, n):
        ap_ps, mb_ps = bcast_rows(rows, n)
        for ci in range(NCH):
            u = ttile([128, 512], "u6", 2)
            V.tensor_tensor(u[:, 0:n], h[:, ci * WH + toff: ci * WH + toff + n],
                            ap_ps[:, 0:n], ALU.mult)
            V.tensor_tensor(u[:, 0:n], u[:, 0:n], mb_ps[:, 0:n], ALU.subtract)
            S.activation(dst_slice(ci, toff, n), u[:, 0:n], AF.Identity,
                         bias=pp[bname][:, pcol + ci:pcol + ci + 1],
                         scale=pp[gname][:, pcol + ci:pcol + ci + 1])

    # =======================================================================
    # Stage 0: per BT4 block: 4-sample sub-blocks -> fp (rfft, channel-major,
    # padded) -> multiscale convs -> h -> LN1(l=0) stats (lag-1 applies).
    # =======================================================================
    rows1 = [None] * 4
    with tc.tile_pool(name="s0f", bufs=1) as s0f:
      fp = [s0f.tile([128, WZ], FP32R, tag=f"fpad{ci}", name=f"fpad{ci}") for ci in range(2)]
      for ci in range(2):
          V.memset(fp[ci][:, :].bitcast(FP32), 0.0)
      with tc.tile_pool(name="s0a", bufs=1) as s0:
        wi = s0.tile([64, H], FP32R, tag="wi", name="wi")
        nc.sync.dma_start(wi[0:64, :], d["w_in"][:, :].bitcast(FP32R))

        def chunked_load(pool, name, dsrc, width, dt=FP32R):
            nch_ = dsrc.shape[0] // 128
            t = pool.tile([128, nch_ * width], dt, tag=name, name=name)
            nc.sync.dma_start(
                t[:, :].rearrange("p (c o) -> p c o", o=width),
                dsrc.rearrange("(c p) o -> p c o", p=128).bitcast(dt) if dt == FP32R
                else dsrc.rearrange("(c p) o -> p c o", p=128))
            return t

        pe = chunked_load(s0, "pe", d["pe_eff"][:, :], T, FP32)
        wsh = chunked_load(s0, "wsh", d["w_shape"][:, :], H)
        wpa = chunked_load(s0, "wpa", d["w_patch"][:, :], H)
        cw = s0.tile([128, 14 * H], FP32R, tag="cw", name="cw")
        conv_taps = {0: [(0, 0)], 1: [(1, 0), (2, 1)],
                     2: [(3, -1), (4, 0), (5, 1), (6, 2)]}

        for bi, (boff, nbs) in enumerate(BT4):
            if bi == 0:
                nc.sync.dma_start(
                    cw[:, :].rearrange("p (k c o) -> p k c o", k=7, c=2),
                    d["convw"][:, :, :].rearrange("k (c p) o -> p k c o", p=128)
                    .bitcast(FP32R))
            for j in range(max(1, nbs // 4)):          # 4-sample sub-blocks
                b0 = boff + 4 * j
                off = b0 * T
                n = 4 * T                              # 384
                xt = s0.tile([64, 384], FP32R, tag="xin", bufs=3, name="xt")
                nc.sync.dma_start(xt[0:64, 0:n], d["xcm"][:, off:off + n].bitcast(FP32R))
                h1b = []
                for co in range(2):
                    hb = s0.tile([128, 384], FP32R, tag="h1b", bufs=4, name="h1b")
                    p = pst([128, 512], "rot", 2)
                    nc.tensor.matmul(p[:, 0:n], wi[0:64, co * 128:(co + 1) * 128],
                                     xt[0:64, 0:n], start=True, stop=True)
                    pe_b = pe[:, co * T:(co + 1) * T].unsqueeze(1).to_broadcast([128, 4, T])
                    V.tensor_tensor(hb[:, 0:n].rearrange("p (b t) -> p b t", t=T),
                                    p[:, 0:n].rearrange("p (b t) -> p b t", t=T),
                                    pe_b, ALU.add)
                    h1b.append(hb)
                sfb = []
                for ci in range(2):
                    sft = s0.tile([128, 384], FP32R, tag="sf", bufs=4, name="sf")
                    V.tensor_tensor(sft[:, 1:n], h1b[ci][:, 1:n],
                                    h1b[ci][:, 0:n - 1], ALU.subtract)
                    V.memset(sft[:, 0:n].rearrange("p (b t) -> p b t", t=T)[:, :, 0:1]
                             .bitcast(FP32), 0.0)
                    sfb.append(sft)
                for co in range(2):
                    p = pst([128, 512], "rot", 2)
                    for ci in range(2):
                        nc.tensor.matmul(p[:, 0:n],
                                         wsh[:, ci * H + co * 128: ci * H + (co + 1) * 128],
                                         sfb[ci][:, 0:n], start=(ci == 0), stop=(ci == 1))
                    V.affine_then_add(h1b[co][:, 0:n], p[:, 0:n], h1b[co][:, 0:n],
                                      1.0, pp["b_shape_pp"][:, co:co + 1])
                # patch (2 groups of 2 samples) + rfft48 (channel-major out)
                for g2 in range(2):
                    hp_ps = pst([128, 384], "aux", 2)
                    nc.tensor.matmul(hp_ps[0:96, 0:H], ones_row[0:1, 0:96], b_patch_row[:, :],
                                     start=True, stop=False)
                    for cp in range(4):
                        p_half, ci = cp // 2, cp % 2
                        lct = s0.tile([128, 96], FP32R, tag="lct", bufs=6, name="lct")
                        S.activation(lct[:, :].rearrange("p (b t) -> p b t", t=T2),
                                     h1b[ci][:, g2 * 192:(g2 + 1) * 192].rearrange(
                                         "p (b t) -> p b t", t=T)[:, :, p_half:T:2]
                                     .bitcast(FP32), AF.Copy)
                        nc.tensor.matmul(hp_ps[0:96, 0:H], lct[:, :], wpa[:, cp * H:(cp + 1) * H],
                                         start=False, stop=(cp == 3))
                    hp_t = s0.tile([96, 256], FP32R, tag="hpt", bufs=3, name="hpt")
                    S.activation(hp_t[0:96, :], hp_ps[0:96, 0:H], AF.Copy)
                    bb0 = b0 + 2 * g2
                    for cc in range(2):
                        fr_ps = pst([128, 128], "t", 1)
                        nc.tensor.matmul(fr_ps[:, 0:50],
                                         hp_t[0:96, cc * 128:(cc + 1) * 128],
                                         c48bd[0:96, 0:50], start=True, stop=True)
                        S.activation(
                            fp[cc][:, 1 + bb0 * 28: 1 + (bb0 + 2) * 28].rearrange(
                                "p (b f) -> p b f", f=28)[:, :, 0:25],
                            fr_ps[:, 0:50].rearrange("p (b f) -> p b f", f=25),
                            AF.Copy)
            # --- LN1(l=0), two blocks behind (chain DVE hides under convs) ---
            if bi == 2:
                rows1[0] = ln_begin(*T500[0])
            if bi == 3:
                rows1[1] = ln_begin(*T500[1])
                apply_tile(zslice, rows1[0], "ln1_g_pp", "ln1_b_pp", 0, *T500[0])
            # --- multiscale convs for this block ---
            n = nbs * 25
            crhs = {}
            for sh_ in (-1, 0, 1, 2):
                for ci in range(2):
                    ct = s0.tile([128, 512], FP32R, tag="crhs", bufs=8, name="crhs")
                    S.activation(ct[:, 0:n].rearrange("p (b f) -> p b f", f=25),
                                 fp[ci][:, 1 + sh_ + boff * 28:][0:128, 0:nbs * 28]
                                 .rearrange("p (b f) -> p b f", f=28)[:, :, 0:25]
                                 .bitcast(FP32), AF.Copy)
                    crhs[(sh_, ci)] = ct
            for co6 in range(NCH):
                m_idx, co_m = co6 // 2, co6 % 2
                taps = conv_taps[m_idx]
                p = pst([128, 512], "rot", 2)
                first = True
                for (tap, sh_) in taps:
                    for ci in range(2):
                        nc.tensor.matmul(p[:, 0:n],
                                         cw[:, (tap * 2 + ci) * H + co_m * 128:
                                            (tap * 2 + ci) * H + (co_m + 1) * 128],
                                         crhs[(sh_, ci)][:, 0:n], start=first,
                                         stop=(tap == taps[-1][0] and ci == 1))
                        first = False
                V.tensor_scalar(h[:, co6 * WH + boff * 25: co6 * WH + boff * 25 + n],
                                p[:, 0:n], pp["conv_b_pp"][:, co6:co6 + 1], None, ALU.add)
        rows1[2] = ln_begin(*T500[2])
        apply_tile(zslice, rows1[1], "ln1_g_pp", "ln1_b_pp", 0, *T500[1])
        rows1[3] = ln_begin(*T500[3])

    ln1_todo = [2, 3]   # LN1 applies still pending at attention start

    # attention constants (emitted after stage0 so their DMA doesn't delay it)
    dft = {}
    for sfx, r, rp, c in (("5", 125, 126, 65), ("4", 100, 100, 52)):
        for nm in ("cqk_r", "cqk_i", "cv_r", "cv_i"):
            dft[nm + sfx] = cload(nm + sfx, [r, c])
        for nm in ("d_r", "d_i"):
            dft[nm + sfx] = cload(nm + sfx, [c, rp])
    bout_pp = cst.tile([NCLS, 1], FP32, tag="bout_pp", name="bout_pp")
    nc.sync.dma_start(bout_pp[0:NCLS, :], d["bout_pp"][:, :])

    # =======================================================================
    # Attention building blocks (tiles in the layer-era `att` pool)
    # =======================================================================
    attp = [None]

    def atile(shape, tag, bufs, dt=FP32):
        return attp[0].tile(list(shape), dt, tag=tag, bufs=bufs, name=f"a_{tag}")

    def emit_qkv(wq, coff, nb):
        ntok = nb * 25
        qk = atile([128, 2 * E], "qs_qk", 2, FP32R)
        qv = atile([128, E], "qs_v", 3, FP32R)
        for (qo, qn) in ((0, 512), (512, 512), (1024, 512), (1536, 512), (2048, 256)):
            p = pst([128, 512], "rot", 2)
            for ci in range(NCH):
                nc.tensor.matmul(p[0:ntok, 0:qn],
                                 zb[:, ci * WZ + coff: ci * WZ + coff + ntok],
                                 wq[ci][:, qo:qo + qn],
                                 start=(ci == 0), stop=(ci == NCH - 1))
            if qo < 2 * E:
                S.activation(qk[0:ntok, qo:qo + qn], p[0:ntok, 0:qn], AF.Copy)
            else:
                S.activation(qv[0:ntok, qo - 2 * E: qo - 2 * E + qn],
                             p[0:ntok, 0:qn], AF.Copy)
        return qk, qv

    def emit_midA(coff, nb, qk, qv):
        """Logits (PE) + logit DVE reduction; returns state for midB."""
        ntok, nfr = nb * 25, nb * 13
        sfx = "5" if nb == 5 else "4"
        lgt = atile([128, 8], "lgt", 2)
        for half in range(2):
            for comp in range(2):
                cmat = dft[("cqk_r" if comp == 0 else "cqk_i") + sfx]
                dps = pst([128, 384], "aux", 2)
                nc.tensor.matmul(dps[0:nfr, :], cmat[0:ntok, 0:nfr],
                                 qk[0:ntok, half * 384: half * 384 + 384],
                                 start=True, stop=True)
                q_s = atile([128, 384], "q_s", 1)
                S.activation(q_s[0:nfr, :], dps[0:nfr, :], AF.Copy)
                kps = pst([128, 384], "aux", 2)
                nc.tensor.matmul(kps[0:nfr, :], cmat[0:ntok, 0:nfr],
                                 qk[0:ntok, E + half * 384: E + half * 384 + 384],
                                 start=True, stop=True)
                prod = atile([128, 384], "prod", 1)
                V.tensor_tensor(prod[0:nfr, :], kps[0:nfr, :], q_s[0:nfr, :], ALU.mult)
                lp = atile([128, 4], "lp", 3)
                V.tensor_reduce(lp[0:nfr, :],
                                prod[0:nfr, :].rearrange("p (h d) -> p h d", h=4),
                                axis=AX.X, op=ALU.add)
                if comp == 0:
                    V.tensor_copy(lgt[0:nfr, half * 4: half * 4 + 4], lp[0:nfr, :])
                else:
                    V.tensor_tensor(lgt[0:nfr, half * 4: half * 4 + 4],
                                    lgt[0:nfr, half * 4: half * 4 + 4],
                                    lp[0:nfr, :], ALU.add)
        return (coff, nb, qv, lgt)

    def emit_midB(st):
        """Softmax transposes + v filter + iDFT to channel-major zb."""
        coff, nb, qv, lgt = st
        ntok, nfr = nb * 25, nb * 13
        sfx = "5" if nb == 5 else "4"
        tps = pst([128, 128], "t", 1)
        nc.tensor.transpose(tps[0:8, 0:nfr], lgt[0:nfr, 0:8], eye[0:nfr, 0:nfr])
        smx = atile([8, 72], "smx", 2)
        V.tensor_copy(smx[0:8, 0:nfr], tps[0:8, 0:nfr])
        mx = atile([8, 8], "mx", 3)
        V.tensor_reduce(mx[0:8, 0:nb],
                        smx[0:8, 0:nfr].rearrange("p (b f) -> p b f", f=13),
                        axis=AX.X, op=ALU.max)
        V.tensor_tensor(smx[0:8, 0:nfr].rearrange("p (b f) -> p b f", f=13),
                        smx[0:8, 0:nfr].rearrange("p (b f) -> p b f", f=13),
                        mx[0:8, 0:nb].unsqueeze(2).to_broadcast([8, nb, 13]),
                        ALU.subtract)
        sme = atile([8, 72], "sme", 2)
        S.activation(sme[0:8, 0:nfr], smx[0:8, 0:nfr], AF.Exp)
        sm_sum = atile([8, 8], "sm_sum", 3)
        V.tensor_reduce(sm_sum[0:8, 0:nb],
                        sme[0:8, 0:nfr].rearrange("p (b f) -> p b f", f=13),
                        axis=AX.X, op=ALU.add)
        sm_rec = atile([8, 8], "sm_rec", 3)
        V.reciprocal(sm_rec[0:8, 0:nb], sm_sum[0:8, 0:nb])
        att_t = atile([8, 72], "att_t", 2, FP32R)
        V.tensor_tensor(att_t[0:8, 0:nfr].rearrange("p (b f) -> p b f", f=13),
                        sme[0:8, 0:nfr].rearrange("p (b f) -> p b f", f=13),
                        sm_rec[0:8, 0:nb].unsqueeze(2).to_broadcast([8, nb, 13]),
                        ALU.mult)
        atps = pst([128, 128], "t", 1)
        nc.tensor.transpose(atps[0:nfr, 0:8], att_t[0:8, 0:nfr].bitcast(FP32),
                            eye[0:8, 0:8])
        att_s = atile([128, 8], "att_s", 2)
        V.tensor_copy(att_s[0:nfr, 0:8], atps[0:nfr, 0:8])

        # v spectral filter (all 4 DFT products first), then iDFT to c-major
        vsc = []
        for half in range(2):
            for comp in range(2):
                cmat = dft[("cv_r" if comp == 0 else "cv_i") + sfx]
                vps = pst([128, 384], "aux", 2)
                nc.tensor.matmul(vps[0:nfr, :], cmat[0:ntok, 0:nfr],
                                 qv[0:ntok, half * 384: half * 384 + 384],
                                 start=True, stop=True)
                vs = atile([128, 384], "vsc", 4, FP32R)
                V.tensor_tensor(vs[0:nfr, :].rearrange("p (h d) -> p h d", h=4),
                                vps[0:nfr, :].rearrange("p (h d) -> p h d", h=4),
                                att_s[0:nfr, half * 4: half * 4 + 4]
                                .unsqueeze(2).to_broadcast([nfr, 4, DH]),
                                ALU.mult)
                vsc.append(vs)
        ntp = ntok + (nb % 2)   # even-padded iDFT width
        for half in range(2):
            ocm = pst([128, 384], "aux", 2)
            for k3 in range(3):
                ci = half * 3 + k3
                nc.tensor.matmul(ocm[:, k3 * ntp: k3 * ntp + ntp],
                                 vsc[2 * half][0:nfr, k3 * 128:(k3 + 1) * 128],
                                 dft["d_r" + sfx][0:nfr, 0:ntp], start=True, stop=False)
                nc.tensor.matmul(ocm[:, k3 * ntp: k3 * ntp + ntp],
                                 vsc[2 * half + 1][0:nfr, k3 * 128:(k3 + 1) * 128],
                                 dft["d_i" + sfx][0:nfr, 0:ntp], start=False, stop=True)
                S.activation(zb[:, ci * WZ + coff: ci * WZ + coff + ntok],
                             ocm[:, k3 * ntp: k3 * ntp + ntok], AF.Copy)

    # =======================================================================
    # Transformer layers
    # =======================================================================
    lp_ctx = ExitStack()
    attp[0] = lp_ctx.enter_context(tc.tile_pool(name="att", bufs=1))
    for l in range(L):
        # ---- attention (qkv(c+1) | midA(c) | midB(c-1) pipeline) ----
        with tc.tile_pool(name=f"wq{l}", bufs=1) as wqp:
            wq = []
            for ci in range(NCH):
                w = wqp.tile([128, 3 * E], FP32R, tag=f"c{ci}", bufs=1, name=f"wq{l}_{ci}")
                wq.append(w)
            for (qo, qn) in ((0, 512), (512, 512), (1024, 512), (1536, 512), (2048, 256)):
                for ci in range(NCH):
                    nc.sync.dma_start(wq[ci][:, qo:qo + qn],
                                      d["wqkv"][l, ci * 128:(ci + 1) * 128,
                                                qo:qo + qn].bitcast(FP32R))
            stA = stB = None
            for ti in range(4):
                for (coff, nb) in TILE_CHUNKS[ti]:
                    qk_c, qv_c = emit_qkv(wq, coff, nb)
                    if ln1_todo:
                        t_ = ln1_todo.pop(0)
                        apply_tile(zslice, rows1[t_], "ln1_g_pp", "ln1_b_pp",
                                   l * NCH, *T500[t_])
                    if stA is not None:
                        stB_new = emit_midA(*stA)
                        if stB is not None:
                            emit_midB(stB)
                        stB = stB_new
                    stA = (coff, nb, qk_c, qv_c)
            stB_new = emit_midA(*stA)
            if stB is not None:
                emit_midB(stB)
            emit_midB(stB_new)

        # ---- Wo GEMM + residual; LN2 stats per tile (lag-1 applies) ----
        with tc.tile_pool(name=f"wo{l}", bufs=1) as wop:
            wo = []
            for ci in range(NCH):
                w = wop.tile([128, E], FP32R, tag=f"c{ci}", bufs=1, name=f"wo{l}_{ci}")
                wo.append(w)
            for sl in range(3):
                for ci in range(NCH):
                    nc.sync.dma_start(wo[ci][:, sl * 256:(sl + 1) * 256],
                                      d["wo"][l, ci * 128:(ci + 1) * 128,
                                              sl * 256:(sl + 1) * 256].bitcast(FP32R))
            rows2 = [None] * 4
            for ti, (toff, n) in enumerate(T500):
                for co in range(NCH):
                    p = pst([128, 512], "rot", 2)
                    for ci in range(NCH):
                        nc.tensor.matmul(p[:, 0:n], wo[ci][:, co * 128:(co + 1) * 128],
                                         zb[:, ci * WZ + toff: ci * WZ + toff + n],
                                         start=(ci == 0), stop=(ci == NCH - 1))
                    V.affine_then_add(h[:, co * WH + toff: co * WH + toff + n], p[:, 0:n],
                                      h[:, co * WH + toff: co * WH + toff + n],
                                      1.0, pp["bo_pp"][:, l * NCH + co: l * NCH + co + 1])
                rows2[ti] = ln_begin(toff, n)
                if ti > 0:
                    apply_tile(zslice, rows2[ti - 1], "ln2_g_pp", "ln2_b_pp",
                               l * NCH, *T500[ti - 1])

        # ---- FFN (bf16 weights streamed in sixths); th5 produces next LN1 ----
        with tc.tile_pool(name=f"ff{l}", bufs=1) as ffp:
            rows_next = [None] * 4
            for th in range(6):
                wf1t, wf2t = [], []
                for ci in range(NCH):
                    w = ffp.tile([128, 512], FP32R, tag=f"w1_{ci}", bufs=2, name=f"wf1_{ci}")
                    wf1t.append(w)
                for ci4 in range(4):
                    w = ffp.tile([128, E], FP32R, tag=f"w2_{ci4}", bufs=2, name=f"wf2_{ci4}")
                    wf2t.append(w)
                for co4 in range(4):
                    for ci in range(NCH):
                        nc.sync.dma_start(
                            wf1t[ci][:, co4 * 128:(co4 + 1) * 128],
                            d["wf1"][l, ci * 128:(ci + 1) * 128,
                                     th * 512 + co4 * 128: th * 512 + (co4 + 1) * 128]
                            .bitcast(FP32R))
                for half in range(2):
                    for ci4 in range(4):
                        nc.sync.dma_start(
                            wf2t[ci4][:, half * 384:(half + 1) * 384],
                            d["wf2"][l, th * 512 + ci4 * 128: th * 512 + (ci4 + 1) * 128,
                                     half * 384:(half + 1) * 384].bitcast(FP32R))
                for ti, (toff, n) in enumerate(T500):
                    gth = ffp.tile([128, 4 * 512], FP32R, tag="gth", bufs=1, name="gth")
                    for co4 in range(4):
                        p = pst([128, 512], "rot", 2)
                        for ci in range(NCH):
                            nc.tensor.matmul(p[:, 0:n], wf1t[ci][:, co4 * 128:(co4 + 1) * 128],
                                             zslice(ci, toff, n),
                                             start=(ci == 0), stop=(ci == NCH - 1))
                        S.activation(gth[:, co4 * 512: co4 * 512 + n], p[:, 0:n], AF.Gelu,
                                     bias=pp["bf1_pp"][:, l * 24 + th * 4 + co4:
                                                       l * 24 + th * 4 + co4 + 1])
                    for co in range(NCH):
                        p2 = pst([128, 512], "rot", 2)
                        for ci4 in range(4):
                            nc.tensor.matmul(p2[:, 0:n], wf2t[ci4][:, co * 128:(co + 1) * 128],
                                             gth[:, ci4 * 512: ci4 * 512 + n],
                                             start=(ci4 == 0), stop=(ci4 == 3))
                        hs = h[:, co * WH + toff: co * WH + toff + n]
                        if th == 0:
                            V.affine_then_add(hs, p2[:, 0:n], hs, 1.0,
                                              pp["bf2_pp"][:, l * NCH + co: l * NCH + co + 1])
                        else:
                            V.tensor_tensor(hs, hs, p2[:, 0:n], ALU.add)
                    if th == 0 and ti == 0:
                        # last LN2 apply, under FFN GEMM cover
                        apply_tile(zslice, rows2[3], "ln2_g_pp", "ln2_b_pp",
                                   l * NCH, *T500[3])
                    if th == 5 and l < L - 1:
                        # next layer's LN1 stats; applies t0 @t1 and t1 @t3
                        # (t1's apply must wait until all zbh reads are done)
                        rows_next[ti] = ln_begin(toff, n)
                        if ti == 1:
                            apply_tile(zslice, rows_next[0], "ln1_g_pp", "ln1_b_pp",
                                       (l + 1) * NCH, *T500[0])
                        if ti == 3:
                            apply_tile(zslice, rows_next[1], "ln1_g_pp", "ln1_b_pp",
                                       (l + 1) * NCH, *T500[1])
            if l < L - 1:
                rows1 = rows_next
                ln1_todo = [2, 3]
    lp_ctx.close()   # free the att pool before the SSM tail

    # =======================================================================
    # SSM conv (3 taps fused in PSUM) + final LN + mean + head
    # =======================================================================
    with tc.tile_pool(name="ssm", bufs=1) as sp:
        wout_sb = sp.tile([128, NCH * NCLS], FP32R, tag="wout", bufs=1, name="wout_sb")
        nc.sync.dma_start(wout_sb[:, :].rearrange("p (c o) -> p c o", o=NCLS),
                          d["wout"][:, :].rearrange("(c p) o -> p c o", p=128).bitcast(FP32R))
        # padded copies of h into zb (per block; DVE overlaps FFN tail)
        for (boff, nbs) in BT4:
            for ci in range(NCH):
                zv = zb[:, ci * WZ + boff * 28: ci * WZ + (boff + nbs) * 28 + 1]
                V.memset(zv[:, 0:1].bitcast(FP32), 0.0)
                V.memset(zv[:, 1:].rearrange("p (b f) -> p b f", f=28)[:, :, 25:28]
                         .bitcast(FP32), 0.0)
                V.tensor_copy(zv[:, 1:].rearrange("p (b f) -> p b f", f=28)[:, :, 0:25],
                              h[:, ci * WH + boff * 25: ci * WH + (boff + nbs) * 25]
                              .rearrange("p (b f) -> p b f", f=25))

        neguv = sp.tile([1, NCLS], FP32R, tag="neguv", bufs=1, name="neguv")
        nc.sync.dma_start(neguv[0:1, :], d["neguv"][:, :].bitcast(FP32R))
        sm_row = sp.tile([1, 64], FP32R, tag="smr", bufs=1, name="sm_row")

        def final_apply_reduce(bi, rows):
            boff, nbs = BT4[bi]
            n = nbs * 25
            ap_ps = pst([128, 512], "sb", 3)
            nc.tensor.matmul(ap_ps[:, 0:n], ones_row[0:1, 0:128],
                             rows[0:1, 0:n], start=True, stop=True)
            with nc.allow_low_precision(reason="f32r store of fp32 sum"):
                V.tensor_reduce(sm_row[0:1, boff:boff + nbs],
                                rows[0:1, 512:512 + n].rearrange(
                                    "p (b f) -> p b f", f=25),
                                axis=AX.X, op=ALU.add)
            for ci in range(NCH):
                u = ttile([128, 512], "u6", 2)
                V.tensor_tensor(u[:, 0:n],
                                h[:, ci * WH + boff * 25: ci * WH + boff * 25 + n],
                                ap_ps[:, 0:n], ALU.mult)
                with nc.allow_low_precision(reason="f32r store of fp32 sum"):
                    V.tensor_reduce(hmean[:, ci * BL + boff: ci * BL + boff + nbs],
                                    u[:, 0:n].rearrange("p (b f) -> p b f", f=25),
                                    axis=AX.X, op=ALU.add)

        # ssm conv: per-tap weight streaming (h += conv(h_pad) + ssm_b)
        rows3 = [None] * 4
        for tap, sh_ in ((0, -1), (1, 0), (2, 1)):
            sw = []
            for ci in range(NCH):
                w = sp.tile([128, E], FP32R, tag=f"swc{ci}", bufs=2, name=f"ssw{ci}")
                nc.sync.dma_start(w[:, :], d["ssmw"][tap, ci * 128:(ci + 1) * 128, :].bitcast(FP32R))
                sw.append(w)
            for bi, (boff, nbs) in enumerate(BT4):
                n = nbs * 25
                crhs = []
                for ci in range(NCH):
                    ct = sp.tile([128, 512], FP32R, tag="scrhs", bufs=8, name="scrhs")
                    S.activation(ct[:, 0:n].rearrange("p (b f) -> p b f", f=25),
                                 zb[:, ci * WZ + 1 + sh_ + boff * 28:][0:128, 0:nbs * 28]
                                 .rearrange("p (b f) -> p b f", f=28)[:, :, 0:25]
                                 .bitcast(FP32), AF.Copy)
                    crhs.append(ct)
                for co in range(NCH):
                    p = pst([128, 512], "rot", 2)
                    for ci in range(NCH):
                        nc.tensor.matmul(p[:, 0:n], sw[ci][:, co * 128:(co + 1) * 128],
                                         crhs[ci][:, 0:n],
                                         start=(ci == 0), stop=(ci == NCH - 1))
                    hs = h[:, co * WH + boff * 25: co * WH + boff * 25 + n]
                    if tap == 0:
                        V.affine_then_add(hs, p[:, 0:n], hs, 1.0, pp["ssmb_pp"][:, co:co + 1])
                    else:
                        V.tensor_tensor(hs, hs, p[:, 0:n], ALU.add)
                if tap == 2:
                    # final LN per finished block, lag-1 apply/reduce
                    rows3[bi] = ln_begin(boff * 25, n)
                    if bi > 0:
                        final_apply_reduce(bi - 1, rows3[bi - 1])
        final_apply_reduce(3, rows3[3])

        hp = pst([128, 128], "rot", 2)
        for ci in range(NCH):
            nc.tensor.matmul(hp[0:NCLS, 0:BL], wout_sb[:, ci * NCLS:(ci + 1) * NCLS],
                             hmean[:, ci * BL:(ci + 1) * BL],
                             start=(ci == 0), stop=False)
        nc.tensor.matmul(hp[0:NCLS, 0:BL], neguv[0:1, 0:NCLS],
                         sm_row[0:1, 0:BL], start=False, stop=True)
        outT = ttile([NCLS, 64], "outT", 1, FP32R)
        V.tensor_scalar(outT[0:NCLS, 0:BL], hp[0:NCLS, 0:BL], bout_pp[0:NCLS, 0:1], None, ALU.add)
        otp = pst([128, 128], "t", 1)
        nc.tensor.transpose(otp[0:BL, 0:NCLS], outT[0:NCLS, 0:BL].bitcast(FP32),
                            eye[0:NCLS, 0:NCLS])
        ofin = ttile([BL, NCLS], "ofin", 1)
        V.tensor_copy(ofin[0:BL, 0:NCLS], otp[0:BL, 0:NCLS])
        nc.sync.dma_start(out_d[:, :], ofin[0:BL, 0:NCLS])


# ---------------------------------------------------------------------------
_NC = None


def _get_nc():
    global _NC
    if _NC is None:
        _NC = _build()
    return _NC


def _run(inputs, trace=False):
    nc = _get_nc()
    sh, xs = _prep(inputs)
    in_maps = [dict(sh, xcm=xs[i]) for i in range(NCORES)]
    res = run_bass_kernel_spmd(nc, in_maps, core_ids=list(range(NCORES)), trace=trace)
    out = np.concatenate([res.results[i]["out"] for i in range(NCORES)], axis=0)
    return out.astype(np.float32), res


def kernel(**inputs):
    out, _ = _run(inputs, trace=False)
    return out
